# revision 6
# baseline (speedup 1.0000x reference)
# RBF Gram matrix kernel for Trainium2 (8 NeuronCores, SPMD).
#
# reference:  G[i, j] = exp(-gamma * ||x_i - y_j||^2)
#                    = exp(2*gamma*((x@y^T)[i,j] - 0.5*||y_j||^2) - gamma*||x_i||^2)
#
# Sharding: row-shard x across 8 cores (1024 rows each), replicate y.
# Each core computes a [1024, 8192] slice of G:
#   PE   : xy = x_c @ y^T       (bf16 in, fp32 PSUM, K=512 as 4 k-tiles)
#   DVE  : s  = xy + (-0.5*||y||^2)  (free-dim broadcast row, one wide op)
#   ACT  : o  = Exp(2*gamma*s + (-gamma*||x||^2))  (per-partition bias)
#   DMA  : o tile (bf16) -> DRAM; host upcasts to fp32
#
# x^T / y^T are shipped pre-permuted into the exact SBUF images so every
# prefetch chunk is one contiguous DMA.
import os

import numpy as np
import ml_dtypes

N_CORES = 8
N_FULL = 8192          # rows of x (and of G)
M_FULL = 8192          # rows of y (cols of G)
D = 512                # feature dim (contraction)
MC = N_FULL // N_CORES # 1024 rows of x per core
P = 128                # SBUF partitions
NT = 512               # moving-dim tile (max; one fp32 psum bank)
KT = D // P            # 4 k-tiles
MT = MC // P           # 8 m-tiles per core
NG = 1024              # psum group width: 2 banks
NGR = M_FULL // NG     # 8 n-groups

_cache = {}


def _build_program(scale2g: float, mc: int, n_full: int, d: int):
    """Build + compile the SPMD bass program. scale2g = 2*gamma immediate."""
    import concourse.mybir as mybir
    import concourse.tile as tile
    from concourse import bacc

    mt = mc // P
    kt = d // P
    ng_w = min(NG, n_full)
    ngroups = n_full // ng_w
    nnsub = ng_w // NT

    nc = bacc.Bacc("TRN2", target_bir_lowering=False, debug=False,
                   num_devices=N_CORES)

    # exact SBUF images (see kernel() for the host-side permutes)
    xT_d = nc.dram_tensor("xTb", [P, mt * kt * P], mybir.dt.bfloat16,
                          kind="ExternalInput").ap()
    yT_d = nc.dram_tensor("yTb", [P, ngroups * kt * ng_w], mybir.dt.bfloat16,
                          kind="ExternalInput").ap()
    y2_d = nc.dram_tensor("y2n", [1, n_full], mybir.dt.float32,
                          kind="ExternalInput").ap()
    x2_d = nc.dram_tensor("x2b", [P, mt], mybir.dt.float32,
                          kind="ExternalInput").ap()
    out_d = nc.dram_tensor("out", [mc, n_full], mybir.dt.bfloat16,
                           kind="ExternalOutput").ap()

    with tile.TileContext(nc) as tc:
        with (
            tc.tile_pool(name="resident", bufs=1) as res_pool,
            tc.tile_pool(name="warm", bufs=1, space="PSUM") as warm_pool,
            tc.tile_pool(name="psum", bufs=3, space="PSUM") as psum_pool,
            tc.tile_pool(name="sq", bufs=4) as s_pool,
            tc.tile_pool(name="ot", bufs=6) as o_pool,
        ):
            xT_sb = res_pool.tile([P, mt * kt * P], mybir.dt.bfloat16, tag="xT")
            yT_sb = res_pool.tile([P, ngroups * kt * ng_w], mybir.dt.bfloat16,
                                  tag="yT")
            y2r_sb = res_pool.tile([1, n_full], mybir.dt.float32, tag="y2r")
            y2_sb = res_pool.tile([P, n_full], mybir.dt.float32, tag="y2")
            x2_sb = res_pool.tile([P, mt], mybir.dt.float32, tag="x2")
            scr_sb = res_pool.tile([P, NT + P], mybir.dt.bfloat16, tag="scr")

            def lhsT(k, m):
                c0 = (m * kt + k) * P
                return xT_sb[:, c0:c0 + P]

            def rhs(k, ng, nn):
                c0 = (ng * kt + k) * ng_w + nn * NT
                return yT_sb[:, c0:c0 + NT]

            # PE warm-up: ~10 matmuls on zeroed scratch keep the HAM
            # activity window busy while the startup DMAs land, so the
            # real matmuls run at 2.4 GHz from the first one.
            nc.vector.memset(scr_sb, 0.0)
            wps = warm_pool.tile([P, NT], mybir.dt.float32)
            for _ in range(10):
                nc.tensor.matmul(wps, lhsT=scr_sb[:, NT:NT + P],
                                 rhs=scr_sb[:, 0:NT], start=True, stop=True)

            # startup set: first xT m-block, norms, first y^T chunk
            nc.sync.dma_start(out=xT_sb[:, 0:kt * P], in_=xT_d[:, 0:kt * P])
            nc.sync.dma_start(out=yT_sb[:, 0:kt * ng_w],
                              in_=yT_d[:, 0:kt * ng_w])
            nc.sync.dma_start(out=x2_sb, in_=x2_d)
            nc.sync.dma_start(out=y2r_sb, in_=y2_d)
            nc.gpsimd.partition_broadcast(y2_sb, y2r_sb[0:1, :])
            nc.sync.dma_start(out=xT_sb[:, kt * P:], in_=xT_d[:, kt * P:])

            def dma_yt_chunk(ch):
                c0 = ch * kt * ng_w
                nc.sync.dma_start(out=yT_sb[:, c0:c0 + kt * ng_w],
                                  in_=yT_d[:, c0:c0 + kt * ng_w])

            if ngroups > 1:
                dma_yt_chunk(1)

            for ng in range(ngroups):
                gsl = slice(ng * ng_w, (ng + 1) * ng_w)
                if ng + 2 < ngroups:
                    dma_yt_chunk(ng + 2)   # just-in-time prefetch
                for m in range(mt):
                    msl = slice(m * P, (m + 1) * P)
                    ps = psum_pool.tile([P, ng_w], mybir.dt.float32)
                    for k in range(kt):
                        for nn in range(nnsub):
                            nc.tensor.matmul(
                                ps[:, nn * NT:(nn + 1) * NT],
                                lhsT=lhsT(k, m),
                                rhs=rhs(k, ng, nn),
                                start=(k == 0),
                                stop=(k == kt - 1),
                            )
                    s = s_pool.tile([P, ng_w], mybir.dt.float32)
                    nc.vector.tensor_add(s, ps, y2_sb[:, gsl])
                    o = o_pool.tile([P, ng_w], mybir.dt.bfloat16)
                    nc.scalar.activation(
                        o, s, mybir.ActivationFunctionType.Exp,
                        bias=x2_sb[:, m:m + 1], scale=float(scale2g),
                    )
                    nc.sync.dma_start(out=out_d[msl, gsl], in_=o)

    nc.compile()
    return nc


def _pack_xT(x_b: np.ndarray) -> np.ndarray:
    """[MC, D] bf16 -> SBUF image [128, MT*KT*128], block (m,k) at col
    (m*KT+k)*128 with element [p, c] = x[m*128 + c, k*128 + p]."""
    mcc, d = x_b.shape
    mt, kt = mcc // P, d // P
    a = x_b.reshape(mt, P, kt, P)          # [m, c, k, p]
    a = a.transpose(3, 0, 2, 1)            # [p, m, k, c]
    return np.ascontiguousarray(a.reshape(P, mt * kt * P))


def _pack_yT(y_b: np.ndarray, ng_w: int) -> np.ndarray:
    """[M, D] bf16 -> SBUF image [128, NGR*KT*ng_w], block (ch,k) at col
    (ch*KT+k)*ng_w with element [p, c] = y[ch*ng_w + c, k*128 + p]."""
    m, d = y_b.shape
    ngr, kt = m // ng_w, d // P
    a = y_b.reshape(ngr, ng_w, kt, P)      # [ch, c, k, p]
    a = a.transpose(3, 0, 2, 1)            # [p, ch, k, c]
    return np.ascontiguousarray(a.reshape(P, ngr * kt * ng_w))


def kernel(x: np.ndarray, y: np.ndarray, gamma: np.ndarray) -> np.ndarray:
    from concourse.bass_utils import run_bass_kernel_spmd

    x = np.asarray(x, dtype=np.float32)
    y = np.asarray(y, dtype=np.float32)
    g = float(np.asarray(gamma))

    n, d = x.shape
    m = y.shape[0]
    assert (n, d, m) == (N_FULL, D, M_FULL), (n, d, m)

    key = (g, n, d, m)
    if key not in _cache:
        _cache.clear()
        _cache[key] = _build_program(2.0 * g, MC, M_FULL, D)
    nc = _cache[key]

    # host-side prep (O(N*D), ~0.01% of kernel FLOPs)
    bf16 = ml_dtypes.bfloat16
    x_b = x.astype(bf16)
    yTb = _pack_yT(y.astype(bf16), NG)
    y2 = np.einsum("md,md->m", y, y, dtype=np.float64)
    y2n = np.ascontiguousarray((-0.5 * y2).astype(np.float32)[None, :])
    x2 = np.einsum("nd,nd->n", x, x, dtype=np.float64)

    in_maps = []
    for c in range(N_CORES):
        sl = slice(c * MC, (c + 1) * MC)
        x2_c = np.ascontiguousarray(
            (-g * x2[sl]).astype(np.float32).reshape(MT, P).T)      # [128, MT]
        in_maps.append({"xTb": _pack_xT(x_b[sl]), "yTb": yTb,
                        "y2n": y2n, "x2b": x2_c})

    trace = bool(int(os.environ.get("RBF_TRACE", "0")))
    res = run_bass_kernel_spmd(nc, in_maps, core_ids=list(range(N_CORES)),
                               trace=trace)
    global LAST_RESULTS
    LAST_RESULTS = res
    return np.concatenate(
        [r["out"].astype(np.float32) for r in res.results], axis=0)


LAST_RESULTS = None


# revision 10
# speedup vs baseline: 1.0743x; 1.0743x over previous
# RBF Gram matrix kernel for Trainium2 (8 NeuronCores, SPMD).
#
# reference:  G[i, j] = exp(-gamma * ||x_i - y_j||^2)
#                    = exp(2*gamma*((x@y^T)[i,j] - 0.5*||y_j||^2) - gamma*||x_i||^2)
#
# Sharding: row-shard x across 8 cores (1024 rows each), replicate y.
# Each core computes a [1024, 8192] slice of G:
#   PE   : xy = x_c @ y^T       (bf16 in, fp32 PSUM, K=512 as 4 k-tiles)
#   DVE  : s  = xy + (-0.5*||y||^2)  (free-dim broadcast row, one wide op)
#   ACT  : o  = Exp(2*gamma*s + (-gamma*||x||^2))  (per-partition bias)
#   DMA  : o tile (bf16) -> DRAM; host upcasts to fp32
#
# x^T / y^T are shipped pre-permuted into the exact SBUF images so every
# prefetch chunk is one contiguous DMA.
import os

import numpy as np
import ml_dtypes

N_CORES = 8
N_FULL = 8192          # rows of x (and of G)
M_FULL = 8192          # rows of y (cols of G)
D = 512                # feature dim (contraction)
MC = N_FULL // N_CORES # 1024 rows of x per core
P = 128                # SBUF partitions
NT = 512               # moving-dim tile (max; one fp32 psum bank)
KT = D // P            # 4 k-tiles
MT = MC // P           # 8 m-tiles per core
NG = 1024              # psum group width: 2 banks
NGR = M_FULL // NG     # 8 n-groups

_cache = {}


def _build_program(scale2g: float, mc: int, n_full: int, d: int):
    """Build + compile the SPMD bass program. scale2g = 2*gamma immediate."""
    import concourse.mybir as mybir
    import concourse.tile as tile
    from concourse import bacc

    mt = mc // P
    kt = d // P
    ng_w = min(NG, n_full)
    ngroups = n_full // ng_w
    nnsub = ng_w // NT

    nc = bacc.Bacc("TRN2", target_bir_lowering=False, debug=False,
                   num_devices=N_CORES)

    # exact SBUF images (see kernel() for the host-side permutes)
    xT_d = nc.dram_tensor("xTb", [P, mt * kt * P], mybir.dt.bfloat16,
                          kind="ExternalInput").ap()
    yT_d = nc.dram_tensor("yTb", [P, ngroups * kt * ng_w], mybir.dt.bfloat16,
                          kind="ExternalInput").ap()
    y2_d = nc.dram_tensor("y2n", [1, n_full], mybir.dt.float32,
                          kind="ExternalInput").ap()
    x2_d = nc.dram_tensor("x2b", [P, mt], mybir.dt.float32,
                          kind="ExternalInput").ap()
    out_d = nc.dram_tensor("out", [mc, n_full], mybir.dt.bfloat16,
                           kind="ExternalOutput").ap()

    with tile.TileContext(nc) as tc:
        with (
            tc.tile_pool(name="resident", bufs=1) as res_pool,
            tc.tile_pool(name="psum", bufs=4, space="PSUM") as psum_pool,
            tc.tile_pool(name="sq", bufs=4) as s_pool,
            tc.tile_pool(name="ot", bufs=6) as o_pool,
        ):
            xT_sb = res_pool.tile([P, mt * kt * P], mybir.dt.bfloat16, tag="xT")
            yT_sb = res_pool.tile([P, ngroups * kt * ng_w], mybir.dt.bfloat16,
                                  tag="yT")
            y2r_sb = res_pool.tile([1, n_full], mybir.dt.float32, tag="y2r")
            y2_sb = res_pool.tile([P, n_full], mybir.dt.float32, tag="y2")
            x2_sb = res_pool.tile([P, mt], mybir.dt.float32, tag="x2")
            scr_sb = res_pool.tile([P, 2 * P], mybir.dt.bfloat16, tag="scr")

            def lhsT(k, m):
                c0 = (m * kt + k) * P
                return xT_sb[:, c0:c0 + P]

            def rhs(k, ng, nn):
                c0 = (ng * kt + k) * ng_w + nn * NT
                return yT_sb[:, c0:c0 + NT]

            # PE warm-up: short matmuls on zeroed scratch keep the HAM
            # activity window busy while the startup DMAs land, so the
            # real matmuls run at 2.4 GHz from the first one. The psum
            # slot is recycled by the pool afterwards.
            nc.vector.memset(scr_sb, 0.0)
            wps = psum_pool.tile([P, ng_w], mybir.dt.float32,
                                 name="wps", tag="ps")
            for _ in range(26):
                nc.tensor.matmul(wps[:, 0:P], lhsT=scr_sb[:, P:2 * P],
                                 rhs=scr_sb[:, 0:P], start=True, stop=True)

            def bcast_y2(ch):
                sl = slice(ch * ng_w, (ch + 1) * ng_w)
                nc.gpsimd.partition_broadcast(y2_sb[:, sl], y2r_sb[0:1, sl])

            def dma_yt_chunk(ch):
                c0 = ch * kt * ng_w
                nc.sync.dma_start(out=yT_sb[:, c0:c0 + kt * ng_w],
                                  in_=yT_d[:, c0:c0 + kt * ng_w])

            # startup set, in critical-path order
            nc.sync.dma_start(out=y2r_sb, in_=y2_d)
            nc.sync.dma_start(out=x2_sb, in_=x2_d)
            nc.sync.dma_start(out=xT_sb[:, 0:kt * P], in_=xT_d[:, 0:kt * P])
            dma_yt_chunk(0)
            bcast_y2(0)
            if mt > 1:
                nc.sync.dma_start(out=xT_sb[:, kt * P:2 * kt * P],
                                  in_=xT_d[:, kt * P:2 * kt * P])
            if mt > 2:
                nc.sync.dma_start(out=xT_sb[:, 2 * kt * P:],
                                  in_=xT_d[:, 2 * kt * P:])
            if ngroups > 1:
                dma_yt_chunk(1)
                bcast_y2(1)

            for ng in range(ngroups):
                gsl = slice(ng * ng_w, (ng + 1) * ng_w)
                if ng + 2 < ngroups:
                    dma_yt_chunk(ng + 2)   # just-in-time prefetch
                    bcast_y2(ng + 2)
                for m in range(mt):
                    msl = slice(m * P, (m + 1) * P)
                    ps = psum_pool.tile([P, ng_w], mybir.dt.float32, tag="ps")
                    for k in range(kt):
                        for nn in range(nnsub):
                            nc.tensor.matmul(
                                ps[:, nn * NT:(nn + 1) * NT],
                                lhsT=lhsT(k, m),
                                rhs=rhs(k, ng, nn),
                                start=(k == 0),
                                stop=(k == kt - 1),
                            )
                    last = (ng == ngroups - 1) and (m == mt - 1)
                    if not last:
                        s = s_pool.tile([P, ng_w], mybir.dt.float32)
                        nc.vector.tensor_add(s, ps, y2_sb[:, gsl])
                        o = o_pool.tile([P, ng_w], mybir.dt.bfloat16)
                        nc.scalar.activation(
                            o, s, mybir.ActivationFunctionType.Exp,
                            bias=x2_sb[:, m:m + 1], scale=float(scale2g),
                        )
                        nc.sync.dma_start(out=out_d[msl, gsl], in_=o)
                    else:
                        # split the final drain chain to shorten the tail
                        for nn in range(nnsub):
                            nsl = slice(ng * ng_w + nn * NT,
                                        ng * ng_w + (nn + 1) * NT)
                            psl = slice(nn * NT, (nn + 1) * NT)
                            s = s_pool.tile([P, NT], mybir.dt.float32,
                                            name=f"sl{nn}", tag=f"sl{nn}")
                            nc.vector.tensor_add(s, ps[:, psl], y2_sb[:, nsl])
                            o = o_pool.tile([P, NT], mybir.dt.bfloat16,
                                            name=f"ol{nn}", tag=f"ol{nn}")
                            nc.scalar.activation(
                                o, s, mybir.ActivationFunctionType.Exp,
                                bias=x2_sb[:, m:m + 1], scale=float(scale2g),
                            )
                            nc.sync.dma_start(out=out_d[msl, nsl], in_=o)

    nc.compile()
    return nc


def _pack_xT(x_b: np.ndarray) -> np.ndarray:
    """[MC, D] bf16 -> SBUF image [128, MT*KT*128], block (m,k) at col
    (m*KT+k)*128 with element [p, c] = x[m*128 + c, k*128 + p]."""
    mcc, d = x_b.shape
    mt, kt = mcc // P, d // P
    a = x_b.reshape(mt, P, kt, P)          # [m, c, k, p]
    a = a.transpose(3, 0, 2, 1)            # [p, m, k, c]
    return np.ascontiguousarray(a.reshape(P, mt * kt * P))


def _pack_yT(y_b: np.ndarray, ng_w: int) -> np.ndarray:
    """[M, D] bf16 -> SBUF image [128, NGR*KT*ng_w], block (ch,k) at col
    (ch*KT+k)*ng_w with element [p, c] = y[ch*ng_w + c, k*128 + p]."""
    m, d = y_b.shape
    ngr, kt = m // ng_w, d // P
    a = y_b.reshape(ngr, ng_w, kt, P)      # [ch, c, k, p]
    a = a.transpose(3, 0, 2, 1)            # [p, ch, k, c]
    return np.ascontiguousarray(a.reshape(P, ngr * kt * ng_w))


def kernel(x: np.ndarray, y: np.ndarray, gamma: np.ndarray) -> np.ndarray:
    from concourse.bass_utils import run_bass_kernel_spmd

    x = np.asarray(x, dtype=np.float32)
    y = np.asarray(y, dtype=np.float32)
    g = float(np.asarray(gamma))

    n, d = x.shape
    m = y.shape[0]
    assert (n, d, m) == (N_FULL, D, M_FULL), (n, d, m)

    key = (g, n, d, m)
    if key not in _cache:
        _cache.clear()
        _cache[key] = _build_program(2.0 * g, MC, M_FULL, D)
    nc = _cache[key]

    # host-side prep (O(N*D), ~0.01% of kernel FLOPs)
    bf16 = ml_dtypes.bfloat16
    x_b = x.astype(bf16)
    yTb = _pack_yT(y.astype(bf16), NG)
    y2 = np.einsum("md,md->m", y, y, dtype=np.float64)
    y2n = np.ascontiguousarray((-0.5 * y2).astype(np.float32)[None, :])
    x2 = np.einsum("nd,nd->n", x, x, dtype=np.float64)

    in_maps = []
    for c in range(N_CORES):
        sl = slice(c * MC, (c + 1) * MC)
        x2_c = np.ascontiguousarray(
            (-g * x2[sl]).astype(np.float32).reshape(MT, P).T)      # [128, MT]
        in_maps.append({"xTb": _pack_xT(x_b[sl]), "yTb": yTb,
                        "y2n": y2n, "x2b": x2_c})

    trace = bool(int(os.environ.get("RBF_TRACE", "0")))
    res = run_bass_kernel_spmd(nc, in_maps, core_ids=list(range(N_CORES)),
                               trace=trace)
    global LAST_RESULTS
    LAST_RESULTS = res
    return np.concatenate(
        [r["out"].astype(np.float32) for r in res.results], axis=0)


LAST_RESULTS = None
